# revision 1
# baseline (speedup 1.0000x reference)
"""Distributed Trainium2 Bass kernel for BrosAttention.

B=2, S=1024, H=768, NH=12, DH=64:
  q,k,v = heads(hidden @ W.T + b)
  scores = q@k^T + einsum('bnid,bijd->bnij', q, bpe)   (bpe = bbox transposed)
  probs  = softmax(scores / 8)
  out    = LN(probs@v @ Wo.T + bo + hidden)

Sharding: 8 cores = 2 batches x 4 query-row blocks of 256 rows. Each core
reads only its 64MB slice of bbox_pos_emb, computes K/V for the full
sequence of its batch (duplicated 4x, cheaper than a collective here), and
writes a disjoint [256, 768] output slice. No collectives.

Layout: transposed scores (scoresT[j, i] per head) because the bias term
q.bpe needs d on partitions; bpe arrives [j, d] and is PE-transposed with
two query rows packed per [128, j] tile. The bias matmul (lhsT = q of one
row as a [64, 12] weight) runs 4 i's concurrently in the four 32-column
groups of the PE array; bias tiles are PE-transposed again into [j, (i,n)]
and added to QK^T psum tiles via a stride-12 AP. Softmax-over-partitions
uses ones-vector matmuls; probs stay unnormalized until after P@V.
"""

import os
import sys
import numpy as np

sys.path.insert(0, "/opt/trn_rl_repo")

B, S, H, NH, DH = 2, 1024, 768, 12, 64
EPS = 1e-12
P = 128
I_CORE = S * B // 8  # 256
N_CORES = 8

_COMPILED = {}


def build_kernel(s=S, i_core=I_CORE, h=H, nh=NH, dh=DH):
    from contextlib import ExitStack
    from concourse import bacc, bass, mybir, tile

    f32 = mybir.dt.float32
    bf16 = mybir.dt.bfloat16
    Alu = mybir.AluOpType
    Act = mybir.ActivationFunctionType
    AxisX = mybir.AxisListType.X

    SC = s // P          # 8 seq chunks
    HC = h // P          # 6 hidden chunks
    IH = i_core // 2     # 128 i's per half
    NDUO_H = IH // 4     # 32 duos per half
    JH = min(512, s)     # fp32 matmul N limit / psum bank
    NJH = s // JH        # 2
    HP = nh // 2         # 6 head pairs
    VH = h // 2          # 384

    nc = bacc.Bacc(None, target_bir_lowering=False, debug=False)

    bf16_ = mybir.dt.bfloat16
    d_hidT = nc.declare_dram_parameter("hidT", [HC, P, s], bf16_, isOutput=False)
    d_hidRT = nc.declare_dram_parameter("hidRT", [HC, P, i_core], bf16_, isOutput=False)
    d_hidR = nc.declare_dram_parameter("hid_rows", [i_core // P, P, h], f32, isOutput=False)
    d_bpe = nc.declare_dram_parameter("bpe", [i_core, dh, s], bf16_, isOutput=False)
    d_W = {w: nc.declare_dram_parameter(w + "T", [HC, P, h], bf16_, isOutput=False)
           for w in ("Wq", "Wk", "Wv", "Wo")}
    d_b = {bn: nc.declare_dram_parameter(bn, [1, h], f32, isOutput=False)
           for bn in ("bq", "bk", "bv", "bo", "ln_gamma", "ln_beta")}
    d_ident = nc.declare_dram_parameter("ident", [P, P], f32, isOutput=False)
    d_out = nc.declare_dram_parameter("out", [i_core // P, P, h], f32, isOutput=True)

    with tile.TileContext(nc) as tc, ExitStack() as ctx:
        # ------------- long-lived pools -------------
        const_p = ctx.enter_context(tc.tile_pool(name="const", bufs=1))
        stat_p = ctx.enter_context(tc.tile_pool(name="stat", bufs=1))
        ps128 = ctx.enter_context(
            tc.tile_pool(name="ps128", bufs=3, space=bass.MemorySpace.PSUM))
        ps512 = ctx.enter_context(
            tc.tile_pool(name="ps512", bufs=1, space=bass.MemorySpace.PSUM))
        psB = ctx.enter_context(
            tc.tile_pool(name="psB", bufs=2, space=bass.MemorySpace.PSUM))
        psS = ctx.enter_context(
            tc.tile_pool(name="psS", bufs=1, space=bass.MemorySpace.PSUM))
        psC = ctx.enter_context(
            tc.tile_pool(name="psC", bufs=1, space=bass.MemorySpace.PSUM))

        # ------------- constants -------------
        ident = const_p.tile([P, P], f32)
        nc.sync.dma_start(ident[:], d_ident[:])
        ones_col = const_p.tile([P, 1], f32)
        nc.vector.memset(ones_col[:], 1.0)
        ones_row = const_p.tile([1, s], f32)
        nc.vector.memset(ones_row[:], 1.0)
        eps_t = const_p.tile([P, 1], f32)
        nc.vector.memset(eps_t[:], EPS)
        zrow = const_p.tile([1, P], bf16)
        nc.vector.memset(zrow[:], 0.0)
        ident_bf = const_p.tile([P, P], bf16)
        nc.vector.tensor_copy(ident_bf[:], ident[:])
        ones_col_bf = const_p.tile([P, 1], bf16)
        nc.vector.memset(ones_col_bf[:], 1.0)
        ones_row_bf = const_p.tile([1, s], bf16)
        nc.vector.memset(ones_row_bf[:], 1.0)
        b_sb = {}
        b_bf = {}
        for bn in ("bq", "bk", "bv", "bo", "ln_gamma", "ln_beta"):
            b_sb[bn] = const_p.tile([1, h], f32, name=f"bias_{bn}")
            nc.sync.dma_start(b_sb[bn][:], d_b[bn][:])
            b_bf[bn] = const_p.tile([1, h], bf16, name=f"biasbf_{bn}")
            nc.vector.tensor_copy(b_bf[bn][:], b_sb[bn][:])

        bcast = {}
        for bn in ("ln_gamma", "ln_beta"):
            t = stat_p.tile([P, h], f32, name=f"bcast_{bn}")
            for c in range(HC):
                pbx = ps128.tile([P, P], f32, name="pt")
                nc.tensor.matmul(pbx[:], ones_row[:, 0:P],
                                 b_sb[bn][:, c * P:(c + 1) * P])
                nc.scalar.copy(t[:, c * P:(c + 1) * P], pbx[:])
            bcast[bn] = t

        # long-lived activations
        hidR = stat_p.tile([P, i_core // P, h], f32)
        nc.sync.dma_start(hidR[:], d_hidR[:].transpose([1, 0, 2]))
        WoT = stat_p.tile([P, HC, h], bf16)
        nc.sync.dma_start(WoT[:], d_W["Wo"][:].transpose([1, 0, 2]))
        qT128 = stat_p.tile([P, nh, i_core], bf16)  # q[n,i,:] at both 64-halves
        qPair = stat_p.tile([P, i_core // 2, 32], bf16)
        kT128 = stat_p.tile([P, HP, s], bf16)
        v_sb = stat_p.tile([P, SC, h], bf16)

        def pe_T(dst_ap, src_ap, copy_eng):
            bf = src_ap.dtype == bf16
            pt = ps128.tile([P, P], bf16 if bf else f32, name="pt")
            n = src_ap.shape[-1]
            nc.tensor.transpose(pt[0:n, :], src_ap,
                                ident_bf[:] if bf else ident[:])
            if copy_eng is nc.scalar:
                copy_eng.copy(dst_ap, pt[0:n, :])
            else:
                copy_eng.tensor_copy(dst_ap, pt[0:n, :])

        # ------------- phase 0 -------------
        with tc.tile_pool(name="early", bufs=1) as early_p:
            hidT = early_p.tile([P, HC, s], bf16)
            nc.sync.dma_start(hidT[:], d_hidT[:].transpose([1, 0, 2]))
            hidRT = early_p.tile([P, HC, i_core], bf16)
            nc.sync.dma_start(hidRT[:], d_hidRT[:].transpose([1, 0, 2]))

            def load_WT(w, dst):
                nc.sync.dma_start(dst[:], d_W[w][:].transpose([1, 0, 2]))
                return dst

            # Q projection (transposed): qT = Wq @ hidR^T + bq
            WqT = load_WT("Wq", early_p.tile([P, HC, h], bf16, name="WT_q"))
            for r in range(HC):
                pq = ps512.tile([P, JH], f32, name="big")
                for kc in range(HC):
                    nc.tensor.matmul(pq[:, 0:i_core],
                                     WqT[:, kc, r * P:(r + 1) * P],
                                     hidRT[:, kc, :], start=(kc == 0), stop=False)
                nc.tensor.matmul(pq[:, 0:i_core], b_bf["bq"][:, r * P:(r + 1) * P],
                                 ones_row_bf[:, 0:i_core], start=False, stop=True)
                for sub in range(2):
                    src = pq[sub * dh:(sub + 1) * dh, 0:i_core]
                    nc.vector.tensor_copy(qT128[0:dh, 2 * r + sub, :], src)
                    nc.vector.tensor_copy(qT128[dh:P, 2 * r + sub, :], src)

            # qPair[k, p, m]: block-diag bias weights: rows 0-63 =
            # q_{2p} in cols 0:12, rows 64-127 = q_{2p+1} in cols 12:24.
            nc.vector.memset(qPair[:], 0.0)
            nc.vector.tensor_copy(
                qPair[0:dh, :, 0:nh],
                qT128[0:dh, :, 0::2].transpose([0, 2, 1]))
            nc.vector.tensor_copy(
                qPair[dh:P, :, nh:2 * nh],
                qT128[dh:P, :, 1::2].transpose([0, 2, 1]))

            # K projection (transposed): kT = Wk @ hid^T + bk
            WkT = load_WT("Wk", early_p.tile([P, HC, h], bf16, name="WT_k"))
            for r in range(HC):
                for jh in range(NJH):
                    pk = ps512.tile([P, JH], f32, name="big")
                    for kc in range(HC):
                        nc.tensor.matmul(pk[:], WkT[:, kc, r * P:(r + 1) * P],
                                         hidT[:, kc, jh * JH:(jh + 1) * JH],
                                         start=(kc == 0), stop=False)
                    nc.tensor.matmul(pk[:], b_bf["bk"][:, r * P:(r + 1) * P],
                                     ones_row_bf[:, 0:JH], start=False, stop=True)
                    nc.vector.tensor_copy(
                        kT128[:, r, jh * JH:(jh + 1) * JH], pk[:])

            # V projection (natural): v = hid @ Wv^T + bv
            WvT = load_WT("Wv", early_p.tile([P, HC, h], bf16, name="WT_v"))
            for jc in range(SC):
                for vh in range(2):
                    pv = ps512.tile([P, JH], f32, name="big")
                    for kc in range(HC):
                        nc.tensor.matmul(pv[:, 0:VH],
                                         hidT[:, kc, jc * P:(jc + 1) * P],
                                         WvT[:, kc, vh * VH:(vh + 1) * VH],
                                         start=(kc == 0), stop=False)
                    nc.tensor.matmul(pv[:, 0:VH], ones_row_bf[:, 0:P],
                                     b_bf["bv"][:, vh * VH:(vh + 1) * VH],
                                     start=False, stop=True)
                    nc.vector.tensor_copy(v_sb[:, jc, vh * VH:(vh + 1) * VH],
                                          pv[:, 0:VH])


        # ------------- phases A+B -------------
        with tc.tile_pool(name="bpeT", bufs=4) as bpeT_p, \
             tc.tile_pool(name="bias4", bufs=1) as bias4_p, \
             tc.tile_pool(name="biasT", bufs=1) as biasT_p, \
             tc.tile_pool(name="sm", bufs=2) as sm_p, \
             tc.tile_pool(name="ctxp", bufs=1) as ctx_p, \
             tc.tile_pool(name="yp", bufs=1) as y_p:
            for half in range(2):
                i0h = half * IH
                # biasT[j, jc, duo*48 + 12*i4 + n]
                biasT = biasT_p.tile([P, SC, NDUO_H * 4 * nh], bf16)

                for octo in range(NDUO_H // 2):
                    pb_h = [psB.tile([P, JH], f32, name="pbh") for j in range(NJH)]
                    for c4 in range(4):
                        pair = octo * 4 + c4
                        iA = i0h + 2 * pair
                        bpeT = bpeT_p.tile([P, s], bf16)
                        nc.sync.dma_start(
                            bpeT[:], d_bpe[iA:iA + 2].rearrange("a b c -> (a b) c"))
                        lhs = qPair[:, (i0h // 2) + pair, 0:32]
                        for jh in range(NJH):
                            nc.tensor.matmul(
                                pb_h[jh][32 * c4:32 * c4 + 32, :], lhs,
                                bpeT[:, jh * JH:(jh + 1) * JH],
                                tile_position=(0, 32 * c4))
                    b4 = bias4_p.tile([P, s], bf16)
                    for jh in range(NJH):
                        nc.vector.tensor_copy(b4[:, jh * JH:(jh + 1) * JH],
                                              pb_h[jh][:])
                    for jc in range(SC):
                        ptb = ps128.tile([P, P], bf16, name="pt")
                        nc.tensor.transpose(ptb[:], b4[:, jc * P:(jc + 1) * P],
                                            ident_bf[:])
                        nc.vector.tensor_copy(
                            biasT[:, jc, octo * 8 * nh:(octo + 1) * 8 * nh
                                  ].rearrange("p (a b) -> p a b", a=4),
                            ptb[:].rearrange("p (a b) -> p a b", a=4)[:, :, 0:2 * nh])

                # ---- attention ----
                ctxT = ctx_p.tile([P, HP, IH], bf16)
                for hp in range(HP):
                    pctx = psC.tile([P, IH], f32, name="pctx")
                    for sub in range(2):
                        n = 2 * hp + sub
                        probsT = sm_p.tile([P, SC, IH], bf16)
                        psum_s = psS.tile([1, IH], f32)
                        for jc in range(SC):
                            pqk = ps128.tile([P, IH], f32, name="pt")
                            sb = sub * dh
                            nc.tensor.matmul(pqk[:],
                                             kT128[sb:sb + dh, hp, jc * P:(jc + 1) * P],
                                             qT128[sb:sb + dh, n, i0h:i0h + IH])
                            sE = sm_p.tile([P, IH], f32)
                            nc.vector.tensor_tensor(
                                sE[:], pqk[:],
                                biasT[:, jc, n:n + nh * (IH - 1) + 1:nh], Alu.add)
                            nc.scalar.activation(probsT[:, jc, :], sE[:],
                                                 Act.Exp, scale=0.125)
                            nc.tensor.matmul(psum_s[:], ones_col_bf[:],
                                             probsT[:, jc, :],
                                             start=(jc == 0), stop=(jc == SC - 1),
                                             skip_group_check=True)
                        rec = sm_p.tile([1, IH], f32)
                        nc.vector.reciprocal(rec[:], psum_s[:])
                        prec = ps128.tile([P, IH], f32, name="pt")
                        nc.tensor.matmul(prec[0:dh, :], ones_row[:, 0:dh], rec[:])
                        recB = sm_p.tile([dh, IH], f32)
                        nc.scalar.copy(recB[:], prec[0:dh, :])
                        for jc in range(SC):
                            nc.tensor.matmul(
                                pctx[sub * dh:(sub + 1) * dh, :],
                                v_sb[:, jc, n * dh:(n + 1) * dh],
                                probsT[:, jc, :],
                                start=(jc == 0), stop=(jc == SC - 1),
                                tile_position=(0, sub * dh),
                                skip_group_check=True)
                        nc.vector.tensor_tensor(
                            pctx[sub * dh:(sub + 1) * dh, :],
                            pctx[sub * dh:(sub + 1) * dh, :],
                            recB[:], Alu.mult)
                    nc.scalar.copy(ctxT[:, hp, :], pctx[:])

                # ---- O-proj + residual + LN ----
                pys = [ps512.tile([P, VH], f32, name="big") for j in range(2)]
                for vh in range(2):
                    for kc in range(HC):
                        nc.tensor.matmul(pys[vh][:], ctxT[:, kc, :],
                                         WoT[:, kc, vh * VH:(vh + 1) * VH],
                                         start=(kc == 0), stop=False)
                    nc.tensor.matmul(pys[vh][:], ones_row_bf[:, 0:P],
                                     b_bf["bo"][:, vh * VH:(vh + 1) * VH],
                                     start=False, stop=True)
                y = y_p.tile([P, h], f32)
                for vh in range(2):
                    nc.vector.tensor_tensor(y[:, vh * VH:(vh + 1) * VH],
                                            pys[vh][:],
                                            hidR[:, half, vh * VH:(vh + 1) * VH],
                                            Alu.add)
                mu = y_p.tile([P, 1], f32)
                nc.vector.tensor_reduce(mu[:], y[:], AxisX, Alu.add)
                nc.vector.tensor_scalar(mu[:], mu[:], 1.0 / h, None, Alu.mult)
                yc = y_p.tile([P, h], f32)
                nc.vector.tensor_scalar(yc[:], y[:], mu[:], None, Alu.subtract)
                ssq = y_p.tile([P, 1], f32)
                nc.scalar.activation(y[:], yc[:], Act.Square, accum_out=ssq[:])
                std = y_p.tile([P, 1], f32)
                nc.scalar.activation(std[:], ssq[:], Act.Sqrt,
                                     scale=1.0 / h, bias=eps_t[:])
                rstd = y_p.tile([P, 1], f32)
                nc.vector.reciprocal(rstd[:], std[:])
                o1 = y_p.tile([P, h], f32)
                nc.vector.tensor_scalar(o1[:], yc[:], rstd[:], None, Alu.mult)
                nc.vector.tensor_tensor(o1[:], o1[:], bcast["ln_gamma"][:], Alu.mult)
                nc.vector.tensor_tensor(o1[:], o1[:], bcast["ln_beta"][:], Alu.add)
                nc.sync.dma_start(d_out[half], o1[:])

    nc.compile()
    return nc


def _shard_inputs(inputs):
    import ml_dtypes
    bf = ml_dtypes.bfloat16
    hs = np.ascontiguousarray(np.asarray(inputs["hidden_states"]), dtype=np.float32)
    bpe = np.asarray(inputs["bbox_pos_emb"])
    ident = np.eye(P, dtype=np.float32)
    # per-batch transposed hidden [H, S] in bf16
    hsT = {b: np.ascontiguousarray(hs[b].T.astype(bf)).reshape(H // P, P, S)
           for b in range(B)}
    WT = {w: np.ascontiguousarray(
             np.asarray(inputs[w], dtype=np.float32).T.astype(bf)).reshape(
                 H // P, P, H)
          for w in ("Wq", "Wk", "Wv", "Wo")}
    in_maps = []
    for c in range(N_CORES):
        b = c // 4
        q0 = (c % 4) * I_CORE
        m = {
            "hidT": hsT[b],
            "hidRT": np.ascontiguousarray(
                hs[b, q0:q0 + I_CORE].T.astype(bf)).reshape(H // P, P, I_CORE),
            "hid_rows": np.ascontiguousarray(
                hs[b, q0:q0 + I_CORE].reshape(I_CORE // P, P, H)),
            "bpe": np.ascontiguousarray(
                bpe[q0:q0 + I_CORE, :, b, :].transpose(0, 2, 1).astype(bf)),
            "ident": ident,
        }
        for w in ("Wq", "Wk", "Wv", "Wo"):
            m[w + "T"] = WT[w]
        for bn in ("bq", "bk", "bv", "bo", "ln_gamma", "ln_beta"):
            m[bn] = np.ascontiguousarray(
                np.asarray(inputs[bn], dtype=np.float32).reshape(1, H))
        in_maps.append(m)
    return in_maps


def _install_ntff_shim():
    """The agent image's antenv lacks axon_hooks; recreate the NTFF profile
    hook via ctypes against libaxon_pjrt.so so trace=True yields
    exec_time_ns + a perfetto trace."""
    import sys as _sys
    if "antenv.axon_hooks" in _sys.modules:
        return
    import types, ctypes, contextlib
    so_path = "/opt/axon/libaxon_pjrt.so"
    mod = types.ModuleType("antenv.axon_hooks")
    _state = {}

    def get_axon_ntff_profile_hook():
        if "hook" in _state:
            return _state["hook"]
        try:
            lib = ctypes.CDLL(so_path)
            if not hasattr(lib, "axon_start_nrt_profile"):
                _state["hook"] = None
                return None
            lib.axon_start_nrt_profile.argtypes = [
                ctypes.POINTER(ctypes.c_int64), ctypes.c_size_t]
            lib.axon_start_nrt_profile.restype = ctypes.c_int64
            lib.axon_stop_nrt_profile.argtypes = [ctypes.c_char_p]
            lib.axon_stop_nrt_profile.restype = ctypes.c_int64
        except OSError:
            _state["hook"] = None
            return None

        @contextlib.contextmanager
        def _hook(output_dir, device_ids):
            import jax
            jax.devices()
            if device_ids:
                ids = (ctypes.c_int64 * len(device_ids))(*device_ids)
                rc = lib.axon_start_nrt_profile(ids, len(device_ids))
            else:
                rc = lib.axon_start_nrt_profile(None, 0)
            if rc != 0:
                raise RuntimeError(f"axon_start_nrt_profile rc={rc}")
            try:
                yield
            finally:
                n = lib.axon_stop_nrt_profile(str(output_dir).encode())
                print(f"ntff profile: {n} file(s) written to {output_dir}")

        _state["hook"] = _hook
        return _hook

    mod.get_axon_ntff_profile_hook = get_axon_ntff_profile_hook
    _sys.modules["antenv.axon_hooks"] = mod


def kernel(**inputs):
    from concourse.bass_utils import run_bass_kernel_spmd

    if os.environ.get("BASS_KERNEL_TRACE"):
        _install_ntff_shim()
        import concourse.bass_utils as _bu
        _bu.upload_artifacts = lambda tmpdir: f"file://{tmpdir}"

    if "nc" not in _COMPILED:
        _COMPILED["nc"] = build_kernel()
    nc = _COMPILED["nc"]
    in_maps = _shard_inputs(inputs)
    res = run_bass_kernel_spmd(nc, in_maps, core_ids=list(range(N_CORES)),
                               trace=bool(os.environ.get("BASS_KERNEL_TRACE")))
    _COMPILED["last_result"] = res
    out = np.zeros((B, S, H), dtype=np.float32)
    for c in range(N_CORES):
        b = c // 4
        q0 = (c % 4) * I_CORE
        out[b, q0:q0 + I_CORE] = np.asarray(
            res.results[c]["out"]).reshape(I_CORE, H)
    return out



# revision 23
# speedup vs baseline: 1.6121x; 1.6121x over previous
"""Distributed Trainium2 Bass kernel for BrosAttention (restructured v2).

B=2, S=1024, H=768, NH=12, DH=64:
  q,k,v = heads(hidden @ W.T + b)
  scores = q@k^T + einsum('bnid,bijd->bnij', q, bpe)   (bpe = bbox transposed)
  probs  = softmax(scores / 8)
  out    = LN(probs@v @ Wo.T + bo + hidden)

Sharding: 8 cores = 2 batches x 4 query-row blocks of 256 rows. Each core
reads only its slice of bbox_pos_emb, computes K/V for the full sequence of
its batch, writes a disjoint [256, 768] output slice. No collectives.

v2 structure (vs v1): transposed scores scoresT[j, (i)] per head; the bias
q.bpe is computed with qPair packed block-diagonally (col order 2n+s) so the
PE-transposed bias tiles are consumed RAW by the score add via strided APs
(no regroup copies). Softmax denominators come out of P@V via a 65th ones-
column on V (no ones-matmul reductions); 1/denom = exp(-ln(denom)) on ACT.
Full i=256 free dims everywhere.
"""

import os
import sys
import numpy as np

sys.path.insert(0, "/opt/trn_rl_repo")

B, S, H, NH, DH = 2, 1024, 768, 12, 64
EPS = 1e-12
P = 128
I_CORE = S * B // 8  # 256
N_CORES = 8

_COMPILED = {}

BPE_DT = "bf16"  # flip to "fp8" for stage 2


def build_kernel(s=S, i_core=I_CORE, h=H, nh=NH, dh=DH):
    from contextlib import ExitStack
    from concourse import bacc, bass, mybir, tile

    f32 = mybir.dt.float32
    bf16 = mybir.dt.bfloat16
    fp8 = mybir.dt.float8e4
    bpe_dt = fp8 if BPE_DT == "fp8" else bf16
    Alu = mybir.AluOpType
    Act = mybir.ActivationFunctionType
    AxisX = mybir.AxisListType.X

    HC = h // P            # 6 hidden chunks
    SC = s // P            # 8 seq chunks (j)
    NPAIR = i_core // 2    # 128 i-pairs
    NOCT = i_core // 8     # 32 octos
    NOG = NOCT // 2        # 16 og-groups (2 octos = 16 i's each)
    NG = nh // 2           # 6 head pairs
    VH = h // 2            # 384

    nc = bacc.Bacc(None, target_bir_lowering=False, debug=False)

    d_hidT = nc.declare_dram_parameter("hidT", [HC, P, s], bf16, isOutput=False)
    d_hidRT = nc.declare_dram_parameter("hidRT", [HC, P, i_core], bf16, isOutput=False)
    d_hidR = nc.declare_dram_parameter("hid_rows", [i_core // P, P, h], f32,
                                       isOutput=False)
    d_bpe = nc.declare_dram_parameter("bpe", [i_core, dh, s], bpe_dt, isOutput=False)
    d_W = {w: nc.declare_dram_parameter(w + "T", [HC, P, h], bf16, isOutput=False)
           for w in ("Wq", "Wk", "Wv", "Wo")}
    d_b = {bn: nc.declare_dram_parameter(bn, [1, h], f32, isOutput=False)
           for bn in ("bq", "bk", "bv", "bo", "ln_gamma", "ln_beta")}
    d_ident = nc.declare_dram_parameter("ident", [P, P], bf16, isOutput=False)
    d_out = nc.declare_dram_parameter("out", [i_core // P, P, h], f32, isOutput=True)

    with tile.TileContext(nc) as tc, ExitStack() as ctx:
        # ---------------- pools ----------------
        const_p = ctx.enter_context(tc.tile_pool(name="const", bufs=1))
        stat_p = ctx.enter_context(tc.tile_pool(name="stat", bufs=1))
        # psum: psA "big" [P,512]f32 x4 bufs (8KB) + ptb 2x2KB + pctx 2x2KB = 16KB
        psA = ctx.enter_context(
            tc.tile_pool(name="psA", bufs=4, space=bass.MemorySpace.PSUM))
        ps2 = ctx.enter_context(
            tc.tile_pool(name="ps2", bufs=2, space=bass.MemorySpace.PSUM))

        def big():
            return psA.tile([P, 512], f32, name="big")
        bpe_p = ctx.enter_context(tc.tile_pool(name="bpe", bufs=2))
        b4_p = ctx.enter_context(tc.tile_pool(name="b4", bufs=2))
        sE_p = ctx.enter_context(tc.tile_pool(name="sE", bufs=3))
        pr_p = ctx.enter_context(tc.tile_pool(name="pr", bufs=3))
        y_p = ctx.enter_context(tc.tile_pool(name="y", bufs=1))

        # ---------------- constants ----------------
        ident_bf = const_p.tile([P, P], bf16)
        nc.sync.dma_start(ident_bf[:], d_ident[:])
        onesP = const_p.tile([P, P], bf16)
        nc.vector.memset(onesP[:], 1.0)
        ones_row = const_p.tile([1, s], bf16)
        nc.vector.memset(ones_row[:], 1.0)
        eps_t = const_p.tile([P, 1], f32)
        nc.vector.memset(eps_t[:], EPS)
        b_sb = {}
        b_bf = {}
        for bn in ("bq", "bk", "bv", "bo", "ln_gamma", "ln_beta"):
            b_sb[bn] = const_p.tile([1, h], f32, name=f"bias_{bn}")
            nc.sync.dma_start(b_sb[bn][:], d_b[bn][:])
            b_bf[bn] = const_p.tile([1, h], bf16, name=f"biasbf_{bn}")
            nc.vector.tensor_copy(b_bf[bn][:], b_sb[bn][:])

        bcast = {}
        for bn in ("ln_gamma", "ln_beta"):
            t = stat_p.tile([P, h], f32, name=f"bcast_{bn}")
            for c in range(HC):
                pbx = big()
                nc.tensor.matmul(pbx[:, 0:P], onesP[0:1, :],
                                 b_bf[bn][:, c * P:(c + 1) * P])
                nc.scalar.copy(t[:, c * P:(c + 1) * P], pbx[:, 0:P])
            bcast[bn] = t

        # ---------------- persistent activations ----------------
        hidR = stat_p.tile([P, i_core // P, h], f32)
        nc.sync.dma_start(hidR[:], d_hidR[:].transpose([1, 0, 2]))
        WoT = stat_p.tile([P, HC, h], bf16)
        nc.sync.dma_start(WoT[:], d_W["Wo"][:].transpose([1, 0, 2]))
        qT128 = stat_p.tile([P, nh, i_core], bf16)   # q[n] duplicated both halves
        qPair = stat_p.tile([P, NPAIR, 32], bf16)    # block-diag, col = 2n+s
        kT128 = stat_p.tile([P, NG, s], bf16)
        v_sb = stat_p.tile([P, SC, nh, dh + 1], bf16)  # col dh = ones
        biasT = stat_p.tile([P, SC, NOCT, 4, 24], bf16)  # raw transposed bias
        ctxT = stat_p.tile([P, NG, i_core], bf16)

        # ---------------- phase E: projections ----------------
        with tc.tile_pool(name="early", bufs=1) as early_p, \
             tc.tile_pool(name="earlyW", bufs=1) as earlyW_p:
            hidT = early_p.tile([P, HC, s], bf16)
            nc.sync.dma_start(hidT[:], d_hidT[:].transpose([1, 0, 2]))
            hidRT = early_p.tile([P, HC, i_core], bf16)
            nc.sync.dma_start(hidRT[:], d_hidRT[:].transpose([1, 0, 2]))

            def load_WT(w):
                t = earlyW_p.tile([P, HC, h], bf16, name="WT")
                nc.sync.dma_start(t[:], d_W[w][:].transpose([1, 0, 2]))
                return t

            # Q projection (transposed): qT = Wq @ hidR^T + bq  -> dup halves
            WqT = load_WT("Wq")
            for r in range(HC):
                pqt = big()
                pq = pqt[:, 0:i_core]
                for kc in range(HC):
                    nc.tensor.matmul(pq, WqT[:, kc, r * P:(r + 1) * P],
                                     hidRT[:, kc, :], start=(kc == 0), stop=False)
                nc.tensor.matmul(pq, b_bf["bq"][:, r * P:(r + 1) * P],
                                 ones_row[:, 0:i_core], start=False, stop=True)
                for sub in range(2):
                    src = pqt[sub * dh:(sub + 1) * dh, 0:i_core]
                    nc.vector.tensor_copy(qT128[0:dh, 2 * r + sub, :], src)
                    nc.vector.tensor_copy(qT128[dh:P, 2 * r + sub, :], src)

            # qPair block-diag: rows 0:64 <- q even-i at cols 2n, rows 64:128 <-
            # q odd-i at cols 2n+1.
            nc.vector.memset(qPair[:], 0.0)
            nc.vector.tensor_copy(
                qPair[0:dh, :, 0:2 * nh:2],
                qT128[0:dh, :, 0::2].transpose([0, 2, 1]))
            nc.vector.tensor_copy(
                qPair[dh:P, :, 1:2 * nh:2],
                qT128[dh:P, :, 1::2].transpose([0, 2, 1]))

            # K projection (transposed): kT = Wk @ hid^T + bk
            WkT = load_WT("Wk")
            for r in range(HC):
                for jh in range(2):
                    pk = big()
                    for kc in range(HC):
                        nc.tensor.matmul(pk[:], WkT[:, kc, r * P:(r + 1) * P],
                                         hidT[:, kc, jh * (s // 2):(jh + 1) * (s // 2)],
                                         start=(kc == 0), stop=False)
                    nc.tensor.matmul(pk[:], b_bf["bk"][:, r * P:(r + 1) * P],
                                     ones_row[:, 0:s // 2], start=False, stop=True)
                    nc.scalar.copy(kT128[:, r, jh * (s // 2):(jh + 1) * (s // 2)],
                                   pk[:])

            # V projection (natural): v = hid @ Wv^T + bv ; 65th column = 1
            WvT = load_WT("Wv")
            for jc in range(SC):
                for vh in range(2):
                    pvt = big()
                    pv = pvt[:, 0:VH]
                    for kc in range(HC):
                        nc.tensor.matmul(pv,
                                         hidT[:, kc, jc * P:(jc + 1) * P],
                                         WvT[:, kc, vh * VH:(vh + 1) * VH],
                                         start=(kc == 0), stop=False)
                    nc.tensor.matmul(pv, ones_row[:, 0:P],
                                     b_bf["bv"][:, vh * VH:(vh + 1) * VH],
                                     start=False, stop=True)
                    nc.scalar.copy(v_sb[:, jc, 6 * vh:6 * vh + 6, 0:dh], pv)
            nc.vector.memset(v_sb[:, :, :, dh:dh + 1], 1.0)

        # ---------------- bias generation ----------------
        # bias[n,i,j] = q[n,i,:].bpe[i,j,:]; computed per i-pair into rows
        # 32*c4 + (2n+s), j streaming; PE-transposed per j-chunk; stored RAW.
        for octo in range(NOCT):
            i0 = octo * 8
            bpeT = bpe_p.tile([P, 4, s], bpe_dt)
            nc.sync.dma_start(
                bpeT[:], d_bpe[i0:i0 + 8].rearrange("(a b) d j -> (b d) a j", a=4))
            pb_h = [big() for _ in range(2)]
            for c4 in range(4):
                lhs = qPair[:, octo * 4 + c4, :]
                for jh in range(2):
                    nc.tensor.matmul(
                        pb_h[jh][32 * c4:32 * c4 + 32, :], lhs,
                        bpeT[:, c4, jh * (s // 2):(jh + 1) * (s // 2)],
                        tile_position=(0, 32 * c4))
            b4 = b4_p.tile([P, s], bf16)
            nc.scalar.copy(b4[:, 0:s // 2], pb_h[0][:])
            nc.vector.tensor_copy(b4[:, s // 2:s], pb_h[1][:])
            ptb = ps2.tile([P, SC, P], bf16, name="ptb")
            for jc in range(SC):
                nc.tensor.transpose(ptb[:, jc, :], b4[:, jc * P:(jc + 1) * P],
                                    ident_bf[:])
            nc.vector.tensor_copy(
                biasT[:, :, octo, :, :],
                ptb[:].rearrange("p j (c u) -> p j c u", c=4)[:, :, :, 0:24])

        # ---------------- attention ----------------
        for g in range(NG):
            pctx = ps2.tile([dh + 1, 2, i_core], f32, name="pctx")
            for jc in range(SC):
                pqk_h = [big() for _ in range(2)]
                for hn in range(2):
                    n = 2 * g + hn
                    bb = dh * (n & 1)
                    nc.tensor.matmul(pqk_h[hn][:, 0:i_core],
                                     kT128[bb:bb + dh, g, jc * P:(jc + 1) * P],
                                     qT128[bb:bb + dh, n, :])
                sE = sE_p.tile([P, 2, i_core], bf16)
                for hn in range(2):
                    n = 2 * g + hn
                    nc.vector.tensor_tensor(
                        sE[:, hn, :].rearrange("p (o c u) -> p o c u", o=NOCT, c=4),
                        pqk_h[hn][:, 0:i_core].rearrange(
                            "p (o c u) -> p o c u", o=NOCT, c=4),
                        biasT[:, jc, :, :, 2 * n:2 * n + 2], Alu.add)
                probsT = pr_p.tile([P, 2, i_core], bf16)
                nc.scalar.activation(probsT[:], sE[:], Act.Exp, scale=0.125)
                for hn in range(2):
                    n = 2 * g + hn
                    nc.tensor.matmul(pctx[:, hn, :], v_sb[:, jc, n, :],
                                     probsT[:, hn, :],
                                     start=(jc == 0), stop=(jc == SC - 1),
                                     skip_group_check=True)
            # evacuate ctx + denominators; 1/denom = exp(-ln(denom)) broadcast
            # to 128 partitions via a K=1 matmul.
            denomS = y_p.tile([1, 2, i_core], bf16, name="denomS")
            for hn in range(2):
                n = 2 * g + hn
                r0 = dh * (n & 1)
                nc.scalar.copy(ctxT[r0:r0 + dh, g, :], pctx[0:dh, hn, :])
                nc.scalar.copy(denomS[:, hn, :], pctx[dh:dh + 1, hn, :])
            prec = big()
            nc.tensor.matmul(prec[:], onesP[0:1, :],
                             denomS[:].rearrange("p a b -> p (a b)"))
            lgr = y_p.tile([P, i_core * 2], f32, name="lgr")
            nc.scalar.activation(lgr[:], prec[:], Act.Ln)
            recB = y_p.tile([P, 2, i_core], bf16, name="recB")
            nc.scalar.activation(recB[:].rearrange("p a b -> p (a b)"), lgr[:],
                                 Act.Exp, scale=-1.0)
            nc.vector.tensor_tensor(ctxT[0:dh, g, :], ctxT[0:dh, g, :],
                                    recB[0:dh, 0, :], Alu.mult)
            nc.vector.tensor_tensor(ctxT[dh:P, g, :], ctxT[dh:P, g, :],
                                    recB[dh:P, 1, :], Alu.mult)

        # ---------------- O-proj + residual + LN ----------------
        for half in range(2):
            i0 = half * P
            pys = [big() for _ in range(2)]
            for vh in range(2):
                for kc in range(HC):
                    nc.tensor.matmul(pys[vh][:, 0:VH], ctxT[:, kc, i0:i0 + P],
                                     WoT[:, kc, vh * VH:(vh + 1) * VH],
                                     start=(kc == 0), stop=False)
                nc.tensor.matmul(pys[vh][:, 0:VH], ones_row[:, 0:P],
                                 b_bf["bo"][:, vh * VH:(vh + 1) * VH],
                                 start=False, stop=True)
            y = y_p.tile([P, h], f32)
            for vh in range(2):
                nc.vector.tensor_tensor(y[:, vh * VH:(vh + 1) * VH],
                                        pys[vh][:, 0:VH],
                                        hidR[:, half, vh * VH:(vh + 1) * VH],
                                        Alu.add)
            mu = y_p.tile([P, 1], f32)
            nc.vector.tensor_reduce(mu[:], y[:], AxisX, Alu.add)
            nc.vector.tensor_scalar(mu[:], mu[:], 1.0 / h, None, Alu.mult)
            yc = y_p.tile([P, h], f32)
            nc.vector.tensor_scalar(yc[:], y[:], mu[:], None, Alu.subtract)
            ssq = y_p.tile([P, 1], f32)
            nc.scalar.activation(y[:], yc[:], Act.Square, accum_out=ssq[:])
            std = y_p.tile([P, 1], f32)
            nc.scalar.activation(std[:], ssq[:], Act.Sqrt,
                                 scale=1.0 / h, bias=eps_t[:])
            rstd = y_p.tile([P, 1], f32)
            nc.vector.reciprocal(rstd[:], std[:])
            o1 = y_p.tile([P, h], f32)
            nc.vector.scalar_tensor_tensor(o1[:], yc[:], rstd[:],
                                           bcast["ln_gamma"][:],
                                           Alu.mult, Alu.mult)
            nc.vector.tensor_tensor(o1[:], o1[:], bcast["ln_beta"][:], Alu.add)
            nc.sync.dma_start(d_out[half], o1[:])

    nc.compile()
    return nc


def _shard_inputs(inputs):
    import ml_dtypes
    bf = ml_dtypes.bfloat16
    bpe_np_dt = ml_dtypes.float8_e4m3 if BPE_DT == "fp8" else bf
    hs = np.ascontiguousarray(np.asarray(inputs["hidden_states"]), dtype=np.float32)
    bpe = np.asarray(inputs["bbox_pos_emb"])
    ident = np.eye(P, dtype=np.float32).astype(bf)
    hsT = {b: np.ascontiguousarray(hs[b].T.astype(bf)).reshape(H // P, P, S)
           for b in range(B)}
    WT = {w: np.ascontiguousarray(
             np.asarray(inputs[w], dtype=np.float32).T.astype(bf)).reshape(
                 H // P, P, H)
          for w in ("Wq", "Wk", "Wv", "Wo")}
    in_maps = []
    for c in range(N_CORES):
        b = c // 4
        q0 = (c % 4) * I_CORE
        m = {
            "hidT": hsT[b],
            "hidRT": np.ascontiguousarray(
                hs[b, q0:q0 + I_CORE].T.astype(bf)).reshape(H // P, P, I_CORE),
            "hid_rows": np.ascontiguousarray(
                hs[b, q0:q0 + I_CORE].reshape(I_CORE // P, P, H)),
            "bpe": np.ascontiguousarray(
                bpe[q0:q0 + I_CORE, :, b, :].transpose(0, 2, 1).astype(bpe_np_dt)),
            "ident": ident,
        }
        for w in ("Wq", "Wk", "Wv", "Wo"):
            m[w + "T"] = WT[w]
        for bn in ("bq", "bk", "bv", "bo", "ln_gamma", "ln_beta"):
            m[bn] = np.ascontiguousarray(
                np.asarray(inputs[bn], dtype=np.float32).reshape(1, H))
        in_maps.append(m)
    return in_maps


def _install_ntff_shim():
    """The agent image's antenv lacks axon_hooks; recreate the NTFF profile
    hook via ctypes against libaxon_pjrt.so so trace=True yields
    exec_time_ns + a perfetto trace."""
    import sys as _sys
    if "antenv.axon_hooks" in _sys.modules:
        return
    import types, ctypes, contextlib
    so_path = "/opt/axon/libaxon_pjrt.so"
    mod = types.ModuleType("antenv.axon_hooks")
    _state = {}

    def get_axon_ntff_profile_hook():
        if "hook" in _state:
            return _state["hook"]
        try:
            lib = ctypes.CDLL(so_path)
            if not hasattr(lib, "axon_start_nrt_profile"):
                _state["hook"] = None
                return None
            lib.axon_start_nrt_profile.argtypes = [
                ctypes.POINTER(ctypes.c_int64), ctypes.c_size_t]
            lib.axon_start_nrt_profile.restype = ctypes.c_int64
            lib.axon_stop_nrt_profile.argtypes = [ctypes.c_char_p]
            lib.axon_stop_nrt_profile.restype = ctypes.c_int64
        except OSError:
            _state["hook"] = None
            return None

        @contextlib.contextmanager
        def _hook(output_dir, device_ids):
            import jax
            jax.devices()
            if device_ids:
                ids = (ctypes.c_int64 * len(device_ids))(*device_ids)
                rc = lib.axon_start_nrt_profile(ids, len(device_ids))
            else:
                rc = lib.axon_start_nrt_profile(None, 0)
            if rc != 0:
                raise RuntimeError(f"axon_start_nrt_profile rc={rc}")
            try:
                yield
            finally:
                n = lib.axon_stop_nrt_profile(str(output_dir).encode())
                print(f"ntff profile: {n} file(s) written to {output_dir}")

        _state["hook"] = _hook
        return _hook

    mod.get_axon_ntff_profile_hook = get_axon_ntff_profile_hook
    _sys.modules["antenv.axon_hooks"] = mod


def kernel(**inputs):
    from concourse.bass_utils import run_bass_kernel_spmd

    if os.environ.get("BASS_KERNEL_TRACE"):
        _install_ntff_shim()
        import concourse.bass_utils as _bu
        _bu.upload_artifacts = lambda tmpdir: f"file://{tmpdir}"

    if "nc" not in _COMPILED:
        _COMPILED["nc"] = build_kernel()
    nc = _COMPILED["nc"]
    in_maps = _shard_inputs(inputs)
    res = run_bass_kernel_spmd(nc, in_maps, core_ids=list(range(N_CORES)),
                               trace=bool(os.environ.get("BASS_KERNEL_TRACE")))
    _COMPILED["last_result"] = res
    out = np.zeros((B, S, H), dtype=np.float32)
    for c in range(N_CORES):
        b = c // 4
        q0 = (c % 4) * I_CORE
        out[b, q0:q0 + I_CORE] = np.asarray(
            res.results[c]["out"]).reshape(I_CORE, H)
    return out


# revision 29
# speedup vs baseline: 2.1199x; 1.3150x over previous
"""Distributed Trainium2 Bass kernel for BrosAttention (restructured v2).

B=2, S=1024, H=768, NH=12, DH=64:
  q,k,v = heads(hidden @ W.T + b)
  scores = q@k^T + einsum('bnid,bijd->bnij', q, bpe)   (bpe = bbox transposed)
  probs  = softmax(scores / 8)
  out    = LN(probs@v @ Wo.T + bo + hidden)

Sharding: 8 cores = 2 batches x 4 query-row blocks of 256 rows. Each core
reads only its slice of bbox_pos_emb, computes K/V for the full sequence of
its batch, writes a disjoint [256, 768] output slice. No collectives.

v2 structure (vs v1): transposed scores scoresT[j, (i)] per head; the bias
q.bpe is computed with qPair packed block-diagonally (col order 2n+s) so the
PE-transposed bias tiles are consumed RAW by the score add via strided APs
(no regroup copies). Softmax denominators come out of P@V via a 65th ones-
column on V (no ones-matmul reductions); 1/denom = exp(-ln(denom)) on ACT.
Full i=256 free dims everywhere.
"""

import os
import sys
import numpy as np

sys.path.insert(0, "/opt/trn_rl_repo")

B, S, H, NH, DH = 2, 1024, 768, 12, 64
EPS = 1e-12
P = 128
I_CORE = S * B // 8  # 256
N_CORES = 8

_COMPILED = {}

BPE_DT = "fp8"


def build_kernel(s=S, i_core=I_CORE, h=H, nh=NH, dh=DH):
    from contextlib import ExitStack
    from concourse import bacc, bass, mybir, tile

    f32 = mybir.dt.float32
    bf16 = mybir.dt.bfloat16
    fp8 = mybir.dt.float8e4
    bpe_dt = fp8 if BPE_DT == "fp8" else bf16
    Alu = mybir.AluOpType
    Act = mybir.ActivationFunctionType
    AxisX = mybir.AxisListType.X

    HC = h // P            # 6 hidden chunks
    SC = s // P            # 8 seq chunks (j)
    NPAIR = i_core // 2    # 128 i-pairs
    NOCT = i_core // 8     # 32 octos
    NOG = NOCT // 2        # 16 og-groups (2 octos = 16 i's each)
    NG = nh // 2           # 6 head pairs
    VH = h // 2            # 384

    nc = bacc.Bacc(None, target_bir_lowering=False, debug=False)

    d_hidT = nc.declare_dram_parameter("hidT", [HC, P, s], bf16, isOutput=False)
    d_hidRT = nc.declare_dram_parameter("hidRT", [HC, P, i_core], bf16, isOutput=False)
    d_hidR = nc.declare_dram_parameter("hid_rows", [i_core // P, P, h], f32,
                                       isOutput=False)
    d_bpe = nc.declare_dram_parameter("bpe", [i_core, dh, s], bpe_dt, isOutput=False)
    d_W = {w: nc.declare_dram_parameter(w + "T", [HC, P, h], bf16, isOutput=False)
           for w in ("Wq", "Wk", "Wv", "Wo")}
    d_b = {bn: nc.declare_dram_parameter(bn, [1, h], f32, isOutput=False)
           for bn in ("bq", "bk", "bv", "bo", "ln_gamma", "ln_beta")}
    d_ident = nc.declare_dram_parameter("ident", [P, P], bf16, isOutput=False)
    d_out = nc.declare_dram_parameter("out", [i_core // P, P, h], f32, isOutput=True)

    with tile.TileContext(nc) as tc, ExitStack() as ctx:
        # ---------------- pools ----------------
        const_p = ctx.enter_context(tc.tile_pool(name="const", bufs=1))
        stat_p = ctx.enter_context(tc.tile_pool(name="stat", bufs=1))
        # psum: psA "big" [P,512]f32 x4 bufs (8KB) + ptb 2x2KB + pctx 2x2KB = 16KB
        psA = ctx.enter_context(
            tc.tile_pool(name="psA", bufs=4, space=bass.MemorySpace.PSUM))
        ps2 = ctx.enter_context(
            tc.tile_pool(name="ps2", bufs=2, space=bass.MemorySpace.PSUM))

        def big():
            return psA.tile([P, 512], f32, name="big")
        bpe_p = ctx.enter_context(tc.tile_pool(name="bpe", bufs=2))
        b4_p = ctx.enter_context(tc.tile_pool(name="b4", bufs=2))
        sE_p = ctx.enter_context(tc.tile_pool(name="sE", bufs=3))
        pr_p = ctx.enter_context(tc.tile_pool(name="pr", bufs=3))
        y_p = ctx.enter_context(tc.tile_pool(name="y", bufs=1))

        # ---------------- constants ----------------
        ident_bf = const_p.tile([P, P], bf16)
        nc.sync.dma_start(ident_bf[:], d_ident[:])
        onesP = const_p.tile([P, P], bf16)
        nc.vector.memset(onesP[:], 1.0)
        ones_row = const_p.tile([1, s], bf16)
        nc.vector.memset(ones_row[:], 1.0)
        eps_t = const_p.tile([P, 1], f32)
        nc.vector.memset(eps_t[:], EPS)
        b_sb = {}
        b_bf = {}
        for bn in ("bq", "bk", "bv", "bo", "ln_gamma", "ln_beta"):
            b_sb[bn] = const_p.tile([1, h], f32, name=f"bias_{bn}")
            nc.sync.dma_start(b_sb[bn][:], d_b[bn][:])
            b_bf[bn] = const_p.tile([1, h], bf16, name=f"biasbf_{bn}")
            nc.vector.tensor_copy(b_bf[bn][:], b_sb[bn][:])

        bcast = {}
        for bn in ("ln_gamma", "ln_beta"):
            t = stat_p.tile([P, h], f32, name=f"bcast_{bn}")
            for c in range(HC):
                pbx = big()
                nc.tensor.matmul(pbx[:, 0:P], onesP[0:1, :],
                                 b_bf[bn][:, c * P:(c + 1) * P])
                nc.scalar.copy(t[:, c * P:(c + 1) * P], pbx[:, 0:P])
            bcast[bn] = t

        # ---------------- persistent activations ----------------
        hidR = stat_p.tile([P, i_core // P, h], f32)
        nc.sync.dma_start(hidR[:], d_hidR[:].transpose([1, 0, 2]))
        WoT = stat_p.tile([P, HC, h], bf16)
        nc.sync.dma_start(WoT[:], d_W["Wo"][:].transpose([1, 0, 2]))
        qT128 = stat_p.tile([P, nh, i_core], bf16)   # q/8 duplicated both halves
        qPair = stat_p.tile([P, NPAIR, 32], bpe_dt)  # block-diag, col = 2n+s
        kT128 = stat_p.tile([P, NG, s], bf16)
        v_sb = stat_p.tile([P, SC, nh, dh + 1], bf16)  # col dh = ones
        biasT = stat_p.tile([P, SC, NOCT, 4, 24], bf16)  # raw transposed bias
        ctxT = stat_p.tile([P, NG, i_core], bf16)
        denomS = stat_p.tile([1, nh, i_core], bf16)

        # ------- phase E (projections) interleaved with bias generation -------
        with tc.tile_pool(name="early", bufs=1) as early_p, \
             tc.tile_pool(name="earlyW", bufs=1) as earlyW_p:
            hidT = early_p.tile([P, HC, s], bf16)
            nc.sync.dma_start(hidT[:], d_hidT[:].transpose([1, 0, 2]))
            hidRT = early_p.tile([P, HC, i_core], bf16)
            nc.sync.dma_start(hidRT[:], d_hidRT[:].transpose([1, 0, 2]))

            def load_WT(w):
                t = earlyW_p.tile([P, HC, h], bf16, name="WT")
                nc.sync.dma_start(t[:], d_W[w][:].transpose([1, 0, 2]))
                return t

            # Q projection (transposed): qT = (Wq @ hidR^T + bq)/8, dup halves.
            # The 1/8 softmax scale is folded into q (QK and bias inherit it).
            WqT = load_WT("Wq")
            for r in range(HC):
                pqt = big()
                pq = pqt[:, 0:i_core]
                for kc in range(HC):
                    nc.tensor.matmul(pq, WqT[:, kc, r * P:(r + 1) * P],
                                     hidRT[:, kc, :], start=(kc == 0), stop=False)
                nc.tensor.matmul(pq, b_bf["bq"][:, r * P:(r + 1) * P],
                                 ones_row[:, 0:i_core], start=False, stop=True)
                for sub in range(2):
                    src = pqt[sub * dh:(sub + 1) * dh, 0:i_core]
                    nc.vector.tensor_scalar(qT128[0:dh, 2 * r + sub, :], src,
                                            0.125, None, Alu.mult)
                    nc.vector.tensor_scalar(qT128[dh:P, 2 * r + sub, :], src,
                                            0.125, None, Alu.mult)

            # qPair block-diag: rows 0:64 <- q even-i at cols 2n, rows 64:128 <-
            # q odd-i at cols 2n+1.
            nc.vector.memset(qPair[:], 0.0)
            nc.vector.tensor_copy(
                qPair[0:dh, :, 0:2 * nh:2],
                qT128[0:dh, :, 0::2].transpose([0, 2, 1]))
            nc.vector.tensor_copy(
                qPair[dh:P, :, 1:2 * nh:2],
                qT128[dh:P, :, 1::2].transpose([0, 2, 1]))
            nc.vector.memset(v_sb[:, :, :, dh:dh + 1], 1.0)

            WkT = load_WT("Wk")
            WvT = load_WT("Wv")

            def k_unit(r, jh):
                pk = big()
                for kc in range(HC):
                    nc.tensor.matmul(pk[:], WkT[:, kc, r * P:(r + 1) * P],
                                     hidT[:, kc, jh * (s // 2):(jh + 1) * (s // 2)],
                                     start=(kc == 0), stop=False)
                nc.tensor.matmul(pk[:], b_bf["bk"][:, r * P:(r + 1) * P],
                                 ones_row[:, 0:s // 2], start=False, stop=True)
                nc.scalar.copy(kT128[:, r, jh * (s // 2):(jh + 1) * (s // 2)],
                               pk[:])

            def v_unit(jc, vh):
                pvt = big()
                pv = pvt[:, 0:VH]
                for kc in range(HC):
                    nc.tensor.matmul(pv,
                                     hidT[:, kc, jc * P:(jc + 1) * P],
                                     WvT[:, kc, vh * VH:(vh + 1) * VH],
                                     start=(kc == 0), stop=False)
                nc.tensor.matmul(pv, ones_row[:, 0:P],
                                 b_bf["bv"][:, vh * VH:(vh + 1) * VH],
                                 start=False, stop=True)
                nc.scalar.copy(v_sb[:, jc, 6 * vh:6 * vh + 6, 0:dh], pv)

            def octo_unit(octo):
                # bias[n,i,j] = q[n,i,:].bpe[i,j,:] into rows 32*c4 + (2n+s),
                # j streaming; PE-transposed per j-chunk; stored RAW (dense).
                i0 = octo * 8
                bpeT = bpe_p.tile([P, 4, s], bpe_dt)
                nc.sync.dma_start(
                    bpeT[:],
                    d_bpe[i0:i0 + 8].rearrange("(a b) d j -> (b d) a j", a=4))
                pb_h = [big() for _ in range(2)]
                for c4 in range(4):
                    lhs = qPair[:, octo * 4 + c4, :]
                    for jh in range(2):
                        nc.tensor.matmul(
                            pb_h[jh][32 * c4:32 * c4 + 32, :], lhs,
                            bpeT[:, c4, jh * (s // 2):(jh + 1) * (s // 2)],
                            tile_position=(0, 32 * c4))
                b4 = b4_p.tile([P, s], bf16)
                nc.scalar.copy(b4[:, 0:s // 2], pb_h[0][:])
                nc.vector.tensor_copy(b4[:, s // 2:s], pb_h[1][:])
                ptb = ps2.tile([P, SC, P], bf16, name="ptb")
                for jc in range(SC):
                    nc.tensor.transpose(ptb[:, jc, :], b4[:, jc * P:(jc + 1) * P],
                                        ident_bf[:])
                src = ptb[:].rearrange("p j (c u) -> p j c u", c=4)[:, :, :, 0:24]
                if octo % 2 == 0:
                    nc.scalar.copy(biasT[:, :, octo, :, :], src)
                else:
                    nc.vector.tensor_copy(biasT[:, :, octo, :, :], src)

            proj_units = ([lambda r=r, jh=jh: k_unit(r, jh)
                           for r in range(HC) for jh in range(2)] +
                          [lambda jc=jc, vh=vh: v_unit(jc, vh)
                           for jc in range(SC) for vh in range(2)])
            pi = 0
            for octo in range(NOCT):
                octo_unit(octo)
                while pi * NOCT < (octo + 1) * len(proj_units):
                    proj_units[pi]()
                    pi += 1

        # ---------------- attention ----------------
        for g in range(NG):
            pctx = ps2.tile([dh + 1, 2, i_core], f32, name="pctx")
            for jc in range(SC):
                pqk_h = [big() for _ in range(2)]
                for hn in range(2):
                    n = 2 * g + hn
                    bb = dh * (n & 1)
                    nc.tensor.matmul(pqk_h[hn][:, 0:i_core],
                                     kT128[bb:bb + dh, g, jc * P:(jc + 1) * P],
                                     qT128[bb:bb + dh, n, :])
                sE = sE_p.tile([P, 2, i_core], bf16)
                for hn in range(2):
                    n = 2 * g + hn
                    nc.vector.tensor_tensor(
                        sE[:, hn, :].rearrange("p (o c u) -> p o c u", o=NOCT, c=4),
                        pqk_h[hn][:, 0:i_core].rearrange(
                            "p (o c u) -> p o c u", o=NOCT, c=4),
                        biasT[:, jc, :, :, 2 * n:2 * n + 2], Alu.add)
                probsT = pr_p.tile([P, 2, i_core], bf16)
                nc.scalar.activation(probsT[:], sE[:], Act.Exp)
                for hn in range(2):
                    n = 2 * g + hn
                    nc.tensor.matmul(pctx[:, hn, :], v_sb[:, jc, n, :],
                                     probsT[:, hn, :],
                                     start=(jc == 0), stop=(jc == SC - 1),
                                     skip_group_check=True)
            # evacuate ctx + denominators (normalization happens in the tail)
            for hn in range(2):
                n = 2 * g + hn
                r0 = dh * (n & 1)
                nc.scalar.copy(ctxT[r0:r0 + dh, g, :], pctx[0:dh, hn, :])
                nc.scalar.copy(denomS[:, n, :], pctx[dh:dh + 1, hn, :])

        # 1/denom = exp(-ln(denom)) broadcast to 128 partitions via K=1 matmuls;
        # Ln and Exp batched to avoid ACT table thrash.
        precs = []
        for g in range(NG):
            prec = big()
            nc.tensor.matmul(prec[:], onesP[0:1, :],
                             denomS[:, 2 * g:2 * g + 2, :].rearrange(
                                 "p a b -> p (a b)"))
            precs.append(prec)
        lgrs = []
        for g in range(NG):
            lgr = y_p.tile([P, i_core * 2], bf16, name=f"lgr{g}")
            nc.scalar.activation(lgr[:], precs[g][:], Act.Ln)
            lgrs.append(lgr)
        for g in range(NG):
            recB = y_p.tile([P, 2, i_core], bf16, name=f"recB{g % 2}")
            nc.scalar.activation(recB[:].rearrange("p a b -> p (a b)"), lgrs[g][:],
                                 Act.Exp, scale=-1.0)
            nc.vector.tensor_tensor(ctxT[0:dh, g, :], ctxT[0:dh, g, :],
                                    recB[0:dh, 0, :], Alu.mult)
            nc.vector.tensor_tensor(ctxT[dh:P, g, :], ctxT[dh:P, g, :],
                                    recB[dh:P, 1, :], Alu.mult)

        # ---------------- O-proj + residual + LN ----------------
        for half in range(2):
            i0 = half * P
            pys = [big() for _ in range(2)]
            for vh in range(2):
                for kc in range(HC):
                    nc.tensor.matmul(pys[vh][:, 0:VH], ctxT[:, kc, i0:i0 + P],
                                     WoT[:, kc, vh * VH:(vh + 1) * VH],
                                     start=(kc == 0), stop=False)
                nc.tensor.matmul(pys[vh][:, 0:VH], ones_row[:, 0:P],
                                 b_bf["bo"][:, vh * VH:(vh + 1) * VH],
                                 start=False, stop=True)
            y = y_p.tile([P, h], f32)
            for vh in range(2):
                nc.vector.tensor_tensor(y[:, vh * VH:(vh + 1) * VH],
                                        pys[vh][:, 0:VH],
                                        hidR[:, half, vh * VH:(vh + 1) * VH],
                                        Alu.add)
            mu = y_p.tile([P, 1], f32)
            nc.vector.tensor_reduce(mu[:], y[:], AxisX, Alu.add)
            nc.vector.tensor_scalar(mu[:], mu[:], 1.0 / h, None, Alu.mult)
            yc = y_p.tile([P, h], f32)
            nc.vector.tensor_scalar(yc[:], y[:], mu[:], None, Alu.subtract)
            ssq = y_p.tile([P, 1], f32)
            nc.scalar.activation(y[:], yc[:], Act.Square, accum_out=ssq[:])
            std = y_p.tile([P, 1], f32)
            nc.scalar.activation(std[:], ssq[:], Act.Sqrt,
                                 scale=1.0 / h, bias=eps_t[:])
            rstd = y_p.tile([P, 1], f32)
            nc.vector.reciprocal(rstd[:], std[:])
            o1 = y_p.tile([P, h], f32)
            nc.vector.scalar_tensor_tensor(o1[:], yc[:], rstd[:],
                                           bcast["ln_gamma"][:],
                                           Alu.mult, Alu.mult)
            nc.vector.tensor_tensor(o1[:], o1[:], bcast["ln_beta"][:], Alu.add)
            nc.sync.dma_start(d_out[half], o1[:])

    nc.compile()
    return nc


def _shard_inputs(inputs):
    import ml_dtypes
    bf = ml_dtypes.bfloat16
    bpe_np_dt = ml_dtypes.float8_e4m3 if BPE_DT == "fp8" else bf
    hs = np.ascontiguousarray(np.asarray(inputs["hidden_states"]), dtype=np.float32)
    bpe = np.asarray(inputs["bbox_pos_emb"])
    ident = np.eye(P, dtype=np.float32).astype(bf)
    hsT = {b: np.ascontiguousarray(hs[b].T.astype(bf)).reshape(H // P, P, S)
           for b in range(B)}
    WT = {w: np.ascontiguousarray(
             np.asarray(inputs[w], dtype=np.float32).T.astype(bf)).reshape(
                 H // P, P, H)
          for w in ("Wq", "Wk", "Wv", "Wo")}
    in_maps = []
    for c in range(N_CORES):
        b = c // 4
        q0 = (c % 4) * I_CORE
        m = {
            "hidT": hsT[b],
            "hidRT": np.ascontiguousarray(
                hs[b, q0:q0 + I_CORE].T.astype(bf)).reshape(H // P, P, I_CORE),
            "hid_rows": np.ascontiguousarray(
                hs[b, q0:q0 + I_CORE].reshape(I_CORE // P, P, H)),
            "bpe": np.ascontiguousarray(
                bpe[q0:q0 + I_CORE, :, b, :].transpose(0, 2, 1).astype(bpe_np_dt)),
            "ident": ident,
        }
        for w in ("Wq", "Wk", "Wv", "Wo"):
            m[w + "T"] = WT[w]
        for bn in ("bq", "bk", "bv", "bo", "ln_gamma", "ln_beta"):
            m[bn] = np.ascontiguousarray(
                np.asarray(inputs[bn], dtype=np.float32).reshape(1, H))
        in_maps.append(m)
    return in_maps


def _install_ntff_shim():
    """The agent image's antenv lacks axon_hooks; recreate the NTFF profile
    hook via ctypes against libaxon_pjrt.so so trace=True yields
    exec_time_ns + a perfetto trace."""
    import sys as _sys
    if "antenv.axon_hooks" in _sys.modules:
        return
    import types, ctypes, contextlib
    so_path = "/opt/axon/libaxon_pjrt.so"
    mod = types.ModuleType("antenv.axon_hooks")
    _state = {}

    def get_axon_ntff_profile_hook():
        if "hook" in _state:
            return _state["hook"]
        try:
            lib = ctypes.CDLL(so_path)
            if not hasattr(lib, "axon_start_nrt_profile"):
                _state["hook"] = None
                return None
            lib.axon_start_nrt_profile.argtypes = [
                ctypes.POINTER(ctypes.c_int64), ctypes.c_size_t]
            lib.axon_start_nrt_profile.restype = ctypes.c_int64
            lib.axon_stop_nrt_profile.argtypes = [ctypes.c_char_p]
            lib.axon_stop_nrt_profile.restype = ctypes.c_int64
        except OSError:
            _state["hook"] = None
            return None

        @contextlib.contextmanager
        def _hook(output_dir, device_ids):
            import jax
            jax.devices()
            if device_ids:
                ids = (ctypes.c_int64 * len(device_ids))(*device_ids)
                rc = lib.axon_start_nrt_profile(ids, len(device_ids))
            else:
                rc = lib.axon_start_nrt_profile(None, 0)
            if rc != 0:
                raise RuntimeError(f"axon_start_nrt_profile rc={rc}")
            try:
                yield
            finally:
                n = lib.axon_stop_nrt_profile(str(output_dir).encode())
                print(f"ntff profile: {n} file(s) written to {output_dir}")

        _state["hook"] = _hook
        return _hook

    mod.get_axon_ntff_profile_hook = get_axon_ntff_profile_hook
    _sys.modules["antenv.axon_hooks"] = mod


def kernel(**inputs):
    from concourse.bass_utils import run_bass_kernel_spmd

    if os.environ.get("BASS_KERNEL_TRACE"):
        _install_ntff_shim()
        import concourse.bass_utils as _bu
        _bu.upload_artifacts = lambda tmpdir: f"file://{tmpdir}"

    if "nc" not in _COMPILED:
        _COMPILED["nc"] = build_kernel()
    nc = _COMPILED["nc"]
    in_maps = _shard_inputs(inputs)
    res = run_bass_kernel_spmd(nc, in_maps, core_ids=list(range(N_CORES)),
                               trace=bool(os.environ.get("BASS_KERNEL_TRACE")))
    _COMPILED["last_result"] = res
    out = np.zeros((B, S, H), dtype=np.float32)
    for c in range(N_CORES):
        b = c // 4
        q0 = (c % 4) * I_CORE
        out[b, q0:q0 + I_CORE] = np.asarray(
            res.results[c]["out"]).reshape(I_CORE, H)
    return out
